# revision 23
# baseline (speedup 1.0000x reference)
"""Trainium2 Bass kernel for autoregressive GMM log-prob (nn_AutoregressiveGMM).

Data-parallel over batch across 8 NeuronCores, fp8 (e4m3) DoubleRow compute.
Per core (B_loc=2048), per step i:
 - first layer: ONE DoubleRow matmul per (n-tile, chunk) using a per-step
   masked stationary S_i = [16*W0x[<i]; 4*I] over a combined moving tensor
   M = [val_fp8; 4*ctxp_fp8] (value masking via host-zeroed stationary rows).
 - residual algebra flattened: t0 = relu(W1[0]h0+b), t1 = relu(W1[1]h0 +
   F t0 + b') with F = W2[0]@W1[1];  h1/h2 never materialize.
 - head: p = Wh_i h0 + G0_i t0 + G1_i t1 + bh_e, G_r = W2[r]@Wh_i
   (precomputed), as col-tiled non-DR fp8 matmuls packing 4 batch chunks
   into one (128,512) psum via tile_position.
 - PE transpose to batch-major; tail (exp/ln GMM math) batched over 8 steps.
 - PSUM->SBUF conversions split between ACT and DVE; post-compile pass
   drops redundant consecutive LDWEIGHTS (stationary reuse across chunks).
"""

import sys

sys.path.insert(0, "/opt/trn_rl_repo")

import numpy as np

import concourse.bass as bass
import concourse.bacc as bacc
import concourse.mybir as mybir
from concourse import tile
from concourse.bass_utils import run_bass_kernel_spmd

B, D, K, H, R, C = 16384, 64, 10, 256, 2, 512
NCORES = 8
BL = B // NCORES          # 2048 rows per core
F32 = mybir.dt.float32
F8 = mybir.dt.float8e4
BF16 = mybir.dt.bfloat16
LOG2PI = float(np.log(2.0 * np.pi))
TB = 8                    # tail batch (steps)


def build_graph():
    nc = bacc.Bacc("TRN2", target_bir_lowering=False, debug=False)
    A = mybir.ActivationFunctionType
    AL = mybir.AluOpType
    DR = mybir.MatmulPerfMode.DoubleRow

    # ---- DRAM parameters ----
    valq_p = nc.declare_dram_parameter("valq", [D, BL], F8, isOutput=False)
    ctxT_p = nc.declare_dram_parameter("ctxT", [C, BL], BF16, isOutput=False)
    w0c_p = nc.declare_dram_parameter("w0c", [C, H], BF16, isOutput=False)
    S_p = nc.declare_dram_parameter("S", [128, D * 2 * 2 * 128], F8, isOutput=False)
    w1a_p = nc.declare_dram_parameter("w1a", [128, 2, H], F8, isOutput=False)
    w1b_p = nc.declare_dram_parameter("w1b", [128, 2, H], F8, isOutput=False)
    fF_p = nc.declare_dram_parameter("fF", [128, 2, H], F8, isOutput=False)
    wh6_p = nc.declare_dram_parameter("wh6", [128, 6, D * 32], F8, isOutput=False)
    cumbT_p = nc.declare_dram_parameter("cumbT", [128, 2 * D], F32, isOutput=False)
    b1aT_p = nc.declare_dram_parameter("b1aT", [128, 2], F32, isOutput=False)
    b1bT_p = nc.declare_dram_parameter("b1bT", [128, 2], F32, isOutput=False)
    bhT_p = nc.declare_dram_parameter("bhT", [128, D], F32, isOutput=False)
    vbm_p = nc.declare_dram_parameter("vbm", [128, 16 * D], F32, isOutput=False)
    out_p = nc.declare_dram_parameter("out", [128, 16], F32, isOutput=True)

    with tile.TileContext(nc) as tc:
        with (
            tc.tile_pool(name="const", bufs=1) as cpool,
            tc.tile_pool(name="state", bufs=2) as spool,
            tc.tile_pool(name="work", bufs=2) as wpool,
            tc.tile_pool(name="ps", bufs=4, space="PSUM") as ppool,
        ):
            # ---- constants ----
            S = cpool.tile([128, D * 2 * 2 * 128], F8, tag="S", name="S")
            nc.sync.dma_start(S[:], S_p[:])
            w1a = cpool.tile([128, 2, H], F8, tag="w1a", name="w1a")
            nc.sync.dma_start(w1a[:], w1a_p[:])
            w1b = cpool.tile([128, 2, H], F8, tag="w1b", name="w1b")
            nc.sync.dma_start(w1b[:], w1b_p[:])
            fF = cpool.tile([128, 2, H], F8, tag="fF", name="fF")
            nc.sync.dma_start(fF[:], fF_p[:])
            wh6 = cpool.tile([128, 6, D * 32], F8, tag="wh6", name="wh6")
            nc.sync.dma_start(wh6[:], wh6_p[:])
            cumbT = cpool.tile([128, 2 * D], F32, tag="cumbT", name="cumbT")
            nc.sync.dma_start(cumbT[:], cumbT_p[:])
            b1aT = cpool.tile([128, 2], F32, tag="b1aT", name="b1aT")
            nc.sync.dma_start(b1aT[:], b1aT_p[:])
            b1bT = cpool.tile([128, 2], F32, tag="b1bT", name="b1bT")
            nc.sync.dma_start(b1bT[:], b1bT_p[:])
            bhT = cpool.tile([128, D], F32, tag="bhT", name="bhT")
            nc.sync.dma_start(bhT[:], bhT_p[:])
            vbm = cpool.tile([128, 16 * D], F32, tag="vbm", name="vbm")
            nc.sync.dma_start(vbm[:], vbm_p[:])

            sumE = cpool.tile([128, D * 16], F32, tag="sumE", name="sumE")
            sumE0 = cpool.tile([128, D * 16], F32, tag="sumE0", name="sumE0")
            c_one = cpool.tile([128, 1], F32, tag="c_one", name="c_one")
            nc.vector.memset(c_one[:], 1.00001)
            c_lhalf = cpool.tile([128, 1], F32, tag="c_lhalf", name="c_lhalf")
            nc.vector.memset(c_lhalf[:], float(np.log(0.5)))

            # combined first-layer moving tensors M[n] = [val; 4*ctxp]
            M = [cpool.tile([128, 2 * BL], F8, tag=f"M{n}", name=f"M{n}")
                 for n in range(2)]
            for n in range(2):
                nc.sync.dma_start(M[n][0:D, 0:BL], valq_p[:])
                nc.vector.memset(M[n][D:128, BL:2 * BL], 0.0)

            # ---- prologue: ctx projection -> fp8 slices of M ----
            with tc.tile_pool(name="ctxload", bufs=1) as ctxpool:
                ctxT = [ctxpool.tile([128, BL], BF16, tag=f"ctxT{k}",
                                     name=f"ctxT{k}") for k in range(4)]
                for k in range(4):
                    nc.sync.dma_start(ctxT[k][:], ctxT_p[128 * k:128 * (k + 1), :])
                w0c = [ctxpool.tile([128, H], BF16, tag=f"w0c{k}",
                                    name=f"w0c{k}") for k in range(4)]
                for k in range(4):
                    nc.sync.dma_start(w0c[k][:], w0c_p[128 * k:128 * (k + 1), :])
                for nf in range(2):
                    for ch in range(4):
                        qf = ppool.tile([128, 1024], F32, tag="q", name="qc")
                        q = qf[:, 0:512]
                        for k in range(4):
                            nc.tensor.matmul(
                                q[:], w0c[k][:, 128 * nf:128 * (nf + 1)],
                                ctxT[k][:, 512 * ch:512 * (ch + 1)],
                                start=(k == 0), stop=(k == 3))
                        # rows 0:64 -> M[nf][64:128, ktile0]; rows 64:128 ->
                        # M[nf][0:64, ktile1]
                        nc.scalar.activation(
                            M[nf][D:128, 512 * ch:512 * (ch + 1)],
                            q[0:D, :], A.Identity, scale=4.0)
                        nc.scalar.activation(
                            M[nf][0:D, BL + 512 * ch:BL + 512 * (ch + 1)],
                            q[D:128, :], A.Identity, scale=4.0)

            # ---- main scan ----
            pTs = None
            for i in range(D):
                h0 = spool.tile([128, 2 * BL], F8, tag="h0", name="h0", bufs=3)
                t0 = spool.tile([128, 2 * BL], F8, tag="t0", name="t0", bufs=3)
                t1 = spool.tile([128, 2 * BL], F8, tag="t1", name="t1", bufs=3)
                h0v = h0[:].rearrange("p (k b) -> p k b", k=2)
                t0v = t0[:].rearrange("p (k b) -> p k b", k=2)
                t1v = t1[:].rearrange("p (k b) -> p k b", k=2)

                # --- h0 = relu(S_i^T M + 16*cumb_i), psum scale 16 ---
                for n in range(2):
                    Sv = S[:, (i * 2 + n) * 256:(i * 2 + n) * 256 + 256] \
                        .rearrange("p (k m) -> p k m", k=2)
                    Mv = M[n][:].rearrange("p (k b) -> p k b", k=2)
                    for cp in range(2):
                        q = ppool.tile([128, 1024], F32, tag="q", name="qh")
                        for cc in range(2):
                            c = 2 * cp + cc
                            nc.tensor.matmul(
                                q[:, 512 * cc:512 * (cc + 1)], Sv,
                                Mv[:, :, 512 * c:512 * (c + 1)],
                                start=True, stop=True, perf_mode=DR)
                        dst = h0[:, n * BL + 1024 * cp:n * BL + 1024 * (cp + 1)]
                        bcol = cumbT[:, n * D + i:n * D + i + 1]
                        if i % 2 == 0:
                            nc.vector.tensor_scalar(dst, q[:], bcol, 0.0,
                                                    op0=AL.add, op1=AL.max)
                        else:
                            nc.scalar.activation(dst, q[:], A.Relu, bias=bcol)

                # --- t0 = relu(w1a^T h0 + 64*b1[0]), psum scale 64 ---
                for n in range(2):
                    Wv = w1a[:, :, 128 * n:128 * (n + 1)]
                    for cp in range(2):
                        q = ppool.tile([128, 1024], F32, tag="q", name="qt0")
                        for cc in range(2):
                            c = 2 * cp + cc
                            nc.tensor.matmul(
                                q[:, 512 * cc:512 * (cc + 1)], Wv,
                                h0v[:, :, 512 * c:512 * (c + 1)],
                                start=True, stop=True, perf_mode=DR)
                        dst = t0[:, n * BL + 1024 * cp:n * BL + 1024 * (cp + 1)]
                        bcol = b1aT[:, n:n + 1]
                        if (n == 0) == (i % 2 == 0):
                            nc.vector.tensor_scalar(dst, q[:], bcol, 0.0,
                                                    op0=AL.add, op1=AL.max)
                        else:
                            nc.scalar.activation(dst, q[:], A.Relu, bias=bcol)

                # --- t1 = relu((w1b^T h0 + F^T t0)/16 + 64*b1e1) scale 1024 ---
                # stationary-major emission: 4-MM runs per weight tile
                for n in range(2):
                    Wv = w1b[:, :, 128 * n:128 * (n + 1)]
                    Fv = fF[:, :, 128 * n:128 * (n + 1)]
                    qs = [ppool.tile([128, 1024], F32, tag="q", name="qt1")
                          for _ in range(2)]
                    for cp in range(2):
                        for cc in range(2):
                            c = 2 * cp + cc
                            nc.tensor.matmul(
                                qs[cp][:, 512 * cc:512 * (cc + 1)], Wv,
                                h0v[:, :, 512 * c:512 * (c + 1)],
                                start=True, stop=False, perf_mode=DR)
                    for cp in range(2):
                        for cc in range(2):
                            c = 2 * cp + cc
                            nc.tensor.matmul(
                                qs[cp][:, 512 * cc:512 * (cc + 1)], Fv,
                                t0v[:, :, 512 * c:512 * (c + 1)],
                                start=False, stop=True, perf_mode=DR)
                    for cp in range(2):
                        dst = t1[:, n * BL + 1024 * cp:n * BL + 1024 * (cp + 1)]
                        bcol = b1bT[:, n:n + 1]
                        if i % 2 == 0:
                            nc.scalar.activation(dst, qs[cp][:], A.Relu,
                                                 bias=bcol)
                        else:
                            nc.vector.tensor_scalar(dst, qs[cp][:], bcol, 0.0,
                                                    op0=AL.add, op1=AL.max)

                # --- head: psum scale 2048, col-tiled into one psum ---
                # t1-sourced k-tiles first so the whole 24-MM block schedules
                # contiguously right after the t1 convs
                ppf = ppool.tile([128, 1024], F32, tag="q", name="pp")
                pp = ppf[:, 0:512]
                srcs = [t1v, t1v, t0v, t0v, h0v, h0v]
                kmap = [4, 5, 2, 3, 0, 1]
                for j6 in range(6):
                    k6 = kmap[j6]
                    wsl = wh6[:, k6, 32 * i:32 * (i + 1)]
                    sv = srcs[j6]
                    kt = k6 % 2
                    for c in range(4):
                        nc.tensor.matmul(
                            pp[32 * c:32 * (c + 1), :], wsl,
                            sv[:, kt, 512 * c:512 * (c + 1)],
                            start=(j6 == 0), stop=(j6 == 5),
                            tile_position=(0, 32 * c))
                psb = wpool.tile([128, 512], BF16, tag="psb", name="psb", bufs=3)
                if i % 2 == 0:
                    nc.scalar.activation(psb[:], pp[:], A.Identity,
                                         bias=bhT[:, i:i + 1],
                                         scale=1.0 / 2048.0)
                else:
                    nc.vector.tensor_scalar(psb[:], pp[:], 1.0 / 2048.0,
                                            bhT[:, i:i + 1],
                                            op0=AL.mult, op1=AL.add)

                # --- transpose to batch-major via DMA xbar, stash in pTs ---
                if i % TB == 0:
                    pTs_prev = pTs if i > 0 else None
                    pTs = spool.tile([128, TB * 512], BF16, tag="pTs",
                                     name="pTs")
                for cb in range(4):
                    nc.sync.dma_start_transpose(
                        pTs[:, 512 * (i % TB) + 128 * cb:
                            512 * (i % TB) + 128 * (cb + 1)],
                        psb[:, 128 * cb:128 * (cb + 1)])

                # --- batched tail, deferred 2 steps past batch end ---
                if i % TB == 1 and i > TB:
                    s0 = i - 1 - TB
                    NG = TB * 16
                    Gv = pTs_prev[:].rearrange("p (g j) -> p g j", j=32)
                    Lv = Gv[:, :, 0:10]
                    Mv_ = Gv[:, :, 10:20]
                    Sv_ = Gv[:, :, 20:30]
                    FD = NG * 10
                    er = lambda t: t[:].rearrange("p (g j) -> p g j", j=10)
                    e0 = wpool.tile([128, FD], F32, tag="e0", name="e0", bufs=1)
                    nc.scalar.activation(er(e0), Lv, A.Exp)
                    nc.vector.tensor_reduce(sumE0[:, 16 * s0:16 * s0 + NG],
                                            er(e0), axis=mybir.AxisListType.X,
                                            op=AL.add)
                    et = wpool.tile([128, FD], F32, tag="et", name="et", bufs=1)
                    nc.scalar.activation(er(et), Sv_, A.Exp)
                    st = wpool.tile([128, FD], F32, tag="st", name="st", bufs=1)
                    nc.scalar.activation(st[:], et[:], A.Ln, bias=c_one[:])
                    lns = wpool.tile([128, FD], F32, tag="lns", name="lns", bufs=1)
                    nc.scalar.activation(lns[:], st[:], A.Ln)
                    inv2 = wpool.tile([128, FD], F32, tag="inv2", name="inv2", bufs=1)
                    nc.scalar.activation(inv2[:], lns[:], A.Exp, scale=-2.0,
                                         bias=c_lhalf[:])
                    dt_ = wpool.tile([128, FD], F32, tag="dt_", name="dt_", bufs=1)
                    vsl = vbm[:, 16 * s0:16 * s0 + NG]
                    nc.gpsimd.tensor_tensor(er(dt_), Mv_,
                                            vsl.to_broadcast((128, NG, 10)),
                                            AL.subtract)
                    sq = wpool.tile([128, FD], F32, tag="sq", name="sq", bufs=1)
                    nc.gpsimd.tensor_tensor(sq[:], dt_[:], dt_[:], AL.mult)
                    w_ = wpool.tile([128, FD], F32, tag="w_", name="w_", bufs=1)
                    nc.gpsimd.tensor_tensor(w_[:], sq[:], inv2[:], AL.mult)
                    u = wpool.tile([128, FD], F32, tag="u", name="u", bufs=1)
                    nc.gpsimd.tensor_tensor(er(u), Lv, er(lns), AL.subtract)
                    tt = wpool.tile([128, FD], F32, tag="tt", name="tt", bufs=1)
                    nc.gpsimd.tensor_tensor(tt[:], u[:], w_[:], AL.subtract)
                    ee = wpool.tile([128, FD], F32, tag="ee", name="ee", bufs=1)
                    nc.scalar.activation(ee[:], tt[:], A.Exp)
                    nc.vector.tensor_reduce(sumE[:, 16 * s0:16 * s0 + NG],
                                            er(ee), axis=mybir.AxisListType.X,
                                            op=AL.add)

            # ---- flush last tail batch ----
            i = D + 1
            s0 = D - TB
            if True:
                if True:
                    NG = TB * 16
                    Gv = pTs[:].rearrange("p (g j) -> p g j", j=32)
                    Lv = Gv[:, :, 0:10]
                    Mv_ = Gv[:, :, 10:20]
                    Sv_ = Gv[:, :, 20:30]
                    FD = NG * 10
                    er = lambda t: t[:].rearrange("p (g j) -> p g j", j=10)
                    e0 = wpool.tile([128, FD], F32, tag="e0", name="e0", bufs=1)
                    nc.scalar.activation(er(e0), Lv, A.Exp)
                    nc.vector.tensor_reduce(sumE0[:, 16 * s0:16 * s0 + NG],
                                            er(e0), axis=mybir.AxisListType.X,
                                            op=AL.add)
                    et = wpool.tile([128, FD], F32, tag="et", name="et", bufs=1)
                    nc.scalar.activation(er(et), Sv_, A.Exp)
                    st = wpool.tile([128, FD], F32, tag="st", name="st", bufs=1)
                    nc.scalar.activation(st[:], et[:], A.Ln, bias=c_one[:])
                    lns = wpool.tile([128, FD], F32, tag="lns", name="lns", bufs=1)
                    nc.scalar.activation(lns[:], st[:], A.Ln)
                    inv2 = wpool.tile([128, FD], F32, tag="inv2", name="inv2", bufs=1)
                    nc.scalar.activation(inv2[:], lns[:], A.Exp, scale=-2.0,
                                         bias=c_lhalf[:])
                    dt_ = wpool.tile([128, FD], F32, tag="dt_", name="dt_", bufs=1)
                    vsl = vbm[:, 16 * s0:16 * s0 + NG]
                    nc.gpsimd.tensor_tensor(er(dt_), Mv_,
                                            vsl.to_broadcast((128, NG, 10)),
                                            AL.subtract)
                    sq = wpool.tile([128, FD], F32, tag="sq", name="sq", bufs=1)
                    nc.gpsimd.tensor_tensor(sq[:], dt_[:], dt_[:], AL.mult)
                    w_ = wpool.tile([128, FD], F32, tag="w_", name="w_", bufs=1)
                    nc.gpsimd.tensor_tensor(w_[:], sq[:], inv2[:], AL.mult)
                    u = wpool.tile([128, FD], F32, tag="u", name="u", bufs=1)
                    nc.gpsimd.tensor_tensor(er(u), Lv, er(lns), AL.subtract)
                    tt = wpool.tile([128, FD], F32, tag="tt", name="tt", bufs=1)
                    nc.gpsimd.tensor_tensor(tt[:], u[:], w_[:], AL.subtract)
                    ee = wpool.tile([128, FD], F32, tag="ee", name="ee", bufs=1)
                    nc.scalar.activation(ee[:], tt[:], A.Exp)
                    nc.vector.tensor_reduce(sumE[:, 16 * s0:16 * s0 + NG],
                                            er(ee), axis=mybir.AxisListType.X,
                                            op=AL.add)

            # ---- finalize ----
            nc.scalar.activation(sumE[:], sumE[:], A.Ln)
            nc.scalar.activation(sumE0[:], sumE0[:], A.Ln)
            nc.vector.tensor_tensor(sumE[:], sumE[:], sumE0[:], AL.subtract)
            acc = cpool.tile([128, 16], F32, tag="acc", name="acc")
            nc.vector.tensor_reduce(
                acc[:], sumE[:].rearrange("p (i g) -> p g i", i=D),
                axis=mybir.AxisListType.X, op=AL.add)
            accf = cpool.tile([128, 16], F32, tag="accf", name="accf")
            nc.vector.tensor_scalar(accf[:], acc[:], -0.5 * LOG2PI * D, None,
                                    op0=AL.add)
            nc.sync.dma_start(out_p[:], accf[:])

    nc.compile()

    # ACT table set consolidation (see baseline comment)
    from concourse.hw_specs import get_activation_tables
    names = list(get_activation_tables(nc.m.arch).keys())
    combined = names.index("natural_log_exp_and_others")
    for b in nc.main_func.blocks:
        keep, first = [], True
        for ins in b.instructions:
            if isinstance(ins, mybir.InstLoadActFuncSet):
                if first:
                    ins.act_func_set_id = combined
                    keep.append(ins)
                    first = False
            else:
                keep.append(ins)
        b.instructions[:] = keep

    # drop redundant consecutive LDWEIGHTS (same stationary reloaded)
    ndedup = 0
    for b in nc.main_func.blocks:
        last_sig = None
        keep = []
        for ins in b.instructions:
            if isinstance(ins, mybir.InstLdweights):
                sig = (repr(ins.ins[0]), repr(ins.perf_mode),
                       repr(ins.is_transpose), repr(ins.tile_position))
                if sig == last_sig:
                    si = ins.sync_info
                    if si is not None and (si.on_wait or si.on_update):
                        keep.append(ins)
                    else:
                        ndedup += 1
                    continue
                last_sig = sig
                keep.append(ins)
            else:
                keep.append(ins)
        b.instructions[:] = keep
    return nc


def prep_inputs(value, context, W0, b0, Wb1, bb1, Wb2, bb2, Wh, bh):
    """Host-side weight prep (fp8 quantization + layouts). Returns in_maps."""
    import ml_dtypes
    f8 = ml_dtypes.float8_e4m3
    bf = ml_dtypes.bfloat16
    f = np.float32

    value = np.asarray(value, f)
    context = np.asarray(context, f)
    W0 = np.asarray(W0, f)
    W0x = W0[:D]                    # (64, 256)
    W0m = W0[D:2 * D]
    W0c = np.ascontiguousarray(W0[2 * D:])
    Wb1 = np.asarray(Wb1, f)
    Wb2 = np.asarray(Wb2, f)
    bb1 = np.asarray(bb1, f)
    bb2 = np.asarray(bb2, f)
    Wh_r = np.asarray(Wh, f).reshape(H, D, 3 * K)
    bh_r = np.asarray(bh, f).reshape(D, 3 * K)

    cum = np.concatenate([np.zeros((1, H), f), np.cumsum(W0m, 0)[:-1]])
    cumb = np.asarray(b0, f)[None, :] + cum          # (64, 256)
    # cumbT[p, n*D + i] = 16*cumb[i, 128n + p]
    cumbT = np.empty((128, 2 * D), f)
    for n in range(2):
        cumbT[:, n * D:(n + 1) * D] = 16.0 * cumb[:, 128 * n:128 * (n + 1)].T

    # first-layer masked stationaries S: [p, ((i*2+n)*2+kt)*128 + m]
    Sm = np.zeros((128, D, 2, 2, 128), f)
    for i in range(D):
        for n in range(2):
            if i > 0:
                Sm[0:i, i, n, 0, :] = 16.0 * W0x[0:i, 128 * n:128 * (n + 1)]
            for qq in range(D):
                Sm[D + qq, i, n, 0, qq] = 4.0       # ctx feats [128n, 128n+64)
                Sm[qq, i, n, 1, D + qq] = 4.0       # ctx feats [128n+64, ..+128)
    S = Sm.reshape(128, D * 2 * 2 * 128).astype(f8)

    def pack2(Wmat, s):
        # (256, 256) -> (128, 2, 256) fp8 scaled: [p, kt, m]
        o = np.empty((128, 2, H), f)
        o[:, 0, :] = Wmat[0:128, :]
        o[:, 1, :] = Wmat[128:256, :]
        return (o * s).astype(f8)

    w1a = pack2(Wb1[0], 4.0)
    w1b = pack2(Wb1[1], 4.0)
    Fm = Wb2[0] @ Wb1[1]
    fF = pack2(Fm, 1.0)

    G0 = np.einsum('hk,kdc->hdc', Wb2[0], Wh_r)      # (256, 64, 30)
    G1 = np.einsum('hk,kdc->hdc', Wb2[1], Wh_r)

    def packhead(Wt, s):
        # (256, D, 30) -> two (128, D*32) blocks scaled
        o = np.zeros((2, 128, D, 32), f)
        o[0, :, :, :30] = Wt[0:128]
        o[1, :, :, :30] = Wt[128:256]
        return (o * s).reshape(2, 128, D * 32)

    wh6 = np.empty((128, 6, D * 32), f)
    wh6[:, 0:2] = packhead(Wh_r, 128.0).transpose(1, 0, 2)
    wh6[:, 2:4] = packhead(G0, 32.0).transpose(1, 0, 2)
    wh6[:, 4:6] = packhead(G1, 32.0).transpose(1, 0, 2)
    wh6 = wh6.astype(f8)

    b1aT = np.empty((128, 2), f)
    b1e1 = bb1[1] + bb2[0] @ Wb1[1]
    b1bT = np.empty((128, 2), f)
    for n in range(2):
        b1aT[:, n] = 64.0 * bb1[0][128 * n:128 * (n + 1)]
        b1bT[:, n] = 64.0 * b1e1[128 * n:128 * (n + 1)]

    cv = bb2[0] + bb2[1]
    bh_e = bh_r + np.einsum("h,hik->ik", cv, Wh_r)   # (64, 30)
    bh_p = np.zeros((D, 32), f)
    bh_p[:, :30] = bh_e
    bhT = np.zeros((128, D), f)
    for ch in range(4):
        bhT[32 * ch:32 * ch + 32, :] = bh_p.T

    W0cb = W0c.astype(bf)

    in_maps = []
    for c in range(NCORES):
        sl = slice(c * BL, (c + 1) * BL)
        vsh = value[sl]
        in_maps.append({
            "valq": np.ascontiguousarray(vsh.T).astype(f8),
            "ctxT": np.ascontiguousarray(context[sl].T).astype(bf),
            "w0c": W0cb, "S": S, "w1a": w1a, "w1b": w1b, "fF": fF,
            "wh6": wh6, "cumbT": cumbT, "b1aT": b1aT, "b1bT": b1bT,
            "bhT": bhT,
            "vbm": np.ascontiguousarray(
                vsh.reshape(4, 4, 128, D).transpose(2, 3, 1, 0)
                .reshape(128, D * 16)),
        })
    return in_maps


def unpack_out(res_list):
    """res[c]['out'] is (128, 16) with col g: b = (g%4)*512 + (g//4)*128 + bp."""
    full = np.empty(B, np.float32)
    for c, r in enumerate(res_list):
        o = np.asarray(r["out"])          # (128, 16)
        shard = o.reshape(128, 4, 4).transpose(2, 1, 0).reshape(BL)
        full[c * BL:(c + 1) * BL] = shard
    return full


_NC_CACHE = {}


def kernel(**inputs):
    if "nc" not in _NC_CACHE:
        _NC_CACHE["nc"] = build_graph()
    nc = _NC_CACHE["nc"]
    in_maps = prep_inputs(**inputs)
    res = run_bass_kernel_spmd(nc, in_maps, core_ids=list(range(NCORES)))
    return unpack_out(res.results)


if __name__ == "__main__":
    np.random.seed(0)
    fake = {
        "value": np.random.randn(B, D).astype(np.float32),
        "context": np.random.randn(B, C).astype(np.float32),
        "W0": (np.random.randn(2 * D + C, H) * 0.02).astype(np.float32),
        "b0": np.zeros(H, np.float32),
        "Wb1": (np.random.randn(R, H, H) * 0.02).astype(np.float32),
        "bb1": np.zeros((R, H), np.float32),
        "Wb2": (np.random.randn(R, H, H) * 0.02).astype(np.float32),
        "bb2": np.zeros((R, H), np.float32),
        "Wh": (np.random.randn(H, 3 * K * D) * 0.02).astype(np.float32),
        "bh": np.zeros(3 * K * D, np.float32),
    }
    out = kernel(**fake)
    print("out", out.shape, out[:4])


# revision 24
# speedup vs baseline: 1.1072x; 1.1072x over previous
"""Trainium2 Bass kernel for autoregressive GMM log-prob (nn_AutoregressiveGMM).

Data-parallel over batch across 8 NeuronCores, fp8 (e4m3) DoubleRow compute.
Per core (B_loc=2048), per step i:
 - first layer: ONE DoubleRow matmul per (n-tile, chunk) using a per-step
   masked stationary S_i = [16*W0x[<i]; 4*I] over a combined moving tensor
   M = [val_fp8; 4*ctxp_fp8] (value masking via host-zeroed stationary rows).
 - residual algebra flattened: t0 = relu(W1[0]h0+b), t1 = relu(W1[1]h0 +
   F t0 + b') with F = W2[0]@W1[1];  h1/h2 never materialize.
 - head: p = Wh_i h0 + G0_i t0 + G1_i t1 + bh_e, G_r = W2[r]@Wh_i
   (precomputed), as col-tiled non-DR fp8 matmuls packing 4 batch chunks
   into one (128,512) psum via tile_position.
 - PE transpose to batch-major; tail (exp/ln GMM math) batched over 8 steps.
 - PSUM->SBUF conversions split between ACT and DVE; post-compile pass
   drops redundant consecutive LDWEIGHTS (stationary reuse across chunks).
"""

import sys

sys.path.insert(0, "/opt/trn_rl_repo")

import numpy as np

import concourse.bass as bass
import concourse.bacc as bacc
import concourse.mybir as mybir
from concourse import tile
from concourse.bass_utils import run_bass_kernel_spmd

B, D, K, H, R, C = 16384, 64, 10, 256, 2, 512
NCORES = 8
BL = B // NCORES          # 2048 rows per core
F32 = mybir.dt.float32
F8 = mybir.dt.float8e4
BF16 = mybir.dt.bfloat16
LOG2PI = float(np.log(2.0 * np.pi))
TB = 8                    # tail batch (steps)


def build_graph():
    nc = bacc.Bacc("TRN2", target_bir_lowering=False, debug=False)
    A = mybir.ActivationFunctionType
    AL = mybir.AluOpType
    DR = mybir.MatmulPerfMode.DoubleRow

    # ---- DRAM parameters ----
    valq_p = nc.declare_dram_parameter("valq", [D, BL], F8, isOutput=False)
    ctxT_p = nc.declare_dram_parameter("ctxT", [C, BL], BF16, isOutput=False)
    w0c_p = nc.declare_dram_parameter("w0c", [C, H], BF16, isOutput=False)
    S_p = nc.declare_dram_parameter("S", [128, D * 2 * 2 * 128], F8, isOutput=False)
    w1a_p = nc.declare_dram_parameter("w1a", [128, 2, H], F8, isOutput=False)
    w1b_p = nc.declare_dram_parameter("w1b", [128, 2, H], F8, isOutput=False)
    fF_p = nc.declare_dram_parameter("fF", [128, 2, H], F8, isOutput=False)
    wh6_p = nc.declare_dram_parameter("wh6", [128, 6, D * 32], F8, isOutput=False)
    cumbT_p = nc.declare_dram_parameter("cumbT", [128, 2 * D], F32, isOutput=False)
    b1aT_p = nc.declare_dram_parameter("b1aT", [128, 2], F32, isOutput=False)
    b1bT_p = nc.declare_dram_parameter("b1bT", [128, 2], F32, isOutput=False)
    bhT_p = nc.declare_dram_parameter("bhT", [128, D], F32, isOutput=False)
    vbm_p = nc.declare_dram_parameter("vbm", [128, 16 * D], F32, isOutput=False)
    out_p = nc.declare_dram_parameter("out", [128, 16], F32, isOutput=True)

    with tile.TileContext(nc) as tc:
        with (
            tc.tile_pool(name="const", bufs=1) as cpool,
            tc.tile_pool(name="state", bufs=2) as spool,
            tc.tile_pool(name="work", bufs=2) as wpool,
            tc.tile_pool(name="ps", bufs=4, space="PSUM") as ppool,
        ):
            # ---- constants ----
            S = cpool.tile([128, D * 2 * 2 * 128], F8, tag="S", name="S")
            nc.sync.dma_start(S[:], S_p[:])
            w1a = cpool.tile([128, 2, H], F8, tag="w1a", name="w1a")
            nc.sync.dma_start(w1a[:], w1a_p[:])
            w1b = cpool.tile([128, 2, H], F8, tag="w1b", name="w1b")
            nc.sync.dma_start(w1b[:], w1b_p[:])
            fF = cpool.tile([128, 2, H], F8, tag="fF", name="fF")
            nc.sync.dma_start(fF[:], fF_p[:])
            wh6 = cpool.tile([128, 6, D * 32], F8, tag="wh6", name="wh6")
            nc.sync.dma_start(wh6[:], wh6_p[:])
            cumbT = cpool.tile([128, 2 * D], F32, tag="cumbT", name="cumbT")
            nc.sync.dma_start(cumbT[:], cumbT_p[:])
            b1aT = cpool.tile([128, 2], F32, tag="b1aT", name="b1aT")
            nc.sync.dma_start(b1aT[:], b1aT_p[:])
            b1bT = cpool.tile([128, 2], F32, tag="b1bT", name="b1bT")
            nc.sync.dma_start(b1bT[:], b1bT_p[:])
            bhT = cpool.tile([128, D], F32, tag="bhT", name="bhT")
            nc.sync.dma_start(bhT[:], bhT_p[:])
            vbm = cpool.tile([128, 16 * D], F32, tag="vbm", name="vbm")
            nc.sync.dma_start(vbm[:], vbm_p[:])

            sumE = cpool.tile([128, D * 16], F32, tag="sumE", name="sumE")
            sumE0 = cpool.tile([128, D * 16], F32, tag="sumE0", name="sumE0")
            c_one = cpool.tile([128, 1], F32, tag="c_one", name="c_one")
            nc.vector.memset(c_one[:], 1.00001)
            c_lhalf = cpool.tile([128, 1], F32, tag="c_lhalf", name="c_lhalf")
            nc.vector.memset(c_lhalf[:], float(np.log(0.5)))

            # combined first-layer moving tensors M[n] = [val; 4*ctxp]
            M = [cpool.tile([128, 2 * BL], F8, tag=f"M{n}", name=f"M{n}")
                 for n in range(2)]
            for n in range(2):
                nc.sync.dma_start(M[n][0:D, 0:BL], valq_p[:])
                nc.vector.memset(M[n][D:128, BL:2 * BL], 0.0)

            # ---- prologue: ctx projection -> fp8 slices of M ----
            with tc.tile_pool(name="ctxload", bufs=1) as ctxpool:
                ctxT = [ctxpool.tile([128, BL], BF16, tag=f"ctxT{k}",
                                     name=f"ctxT{k}") for k in range(4)]
                for k in range(4):
                    nc.sync.dma_start(ctxT[k][:], ctxT_p[128 * k:128 * (k + 1), :])
                w0c = [ctxpool.tile([128, H], BF16, tag=f"w0c{k}",
                                    name=f"w0c{k}") for k in range(4)]
                for k in range(4):
                    nc.sync.dma_start(w0c[k][:], w0c_p[128 * k:128 * (k + 1), :])
                for nf in range(2):
                    for ch in range(4):
                        qf = ppool.tile([128, 1024], F32, tag="q", name="qc")
                        q = qf[:, 0:512]
                        for k in range(4):
                            nc.tensor.matmul(
                                q[:], w0c[k][:, 128 * nf:128 * (nf + 1)],
                                ctxT[k][:, 512 * ch:512 * (ch + 1)],
                                start=(k == 0), stop=(k == 3))
                        # rows 0:64 -> M[nf][64:128, ktile0]; rows 64:128 ->
                        # M[nf][0:64, ktile1]
                        nc.scalar.activation(
                            M[nf][D:128, 512 * ch:512 * (ch + 1)],
                            q[0:D, :], A.Identity, scale=4.0)
                        nc.scalar.activation(
                            M[nf][0:D, BL + 512 * ch:BL + 512 * (ch + 1)],
                            q[D:128, :], A.Identity, scale=4.0)

            # ---- main scan ----
            pTs = None
            for i in range(D):
                h0 = spool.tile([128, 2 * BL], F8, tag="h0", name="h0", bufs=3)
                t0 = spool.tile([128, 2 * BL], F8, tag="t0", name="t0", bufs=3)
                t1 = spool.tile([128, 2 * BL], F8, tag="t1", name="t1", bufs=3)
                h0v = h0[:].rearrange("p (k b) -> p k b", k=2)
                t0v = t0[:].rearrange("p (k b) -> p k b", k=2)
                t1v = t1[:].rearrange("p (k b) -> p k b", k=2)

                # --- h0 = relu(S_i^T M + 16*cumb_i), psum scale 16 ---
                for n in range(2):
                    Sv = S[:, (i * 2 + n) * 256:(i * 2 + n) * 256 + 256] \
                        .rearrange("p (k m) -> p k m", k=2)
                    Mv = M[n][:].rearrange("p (k b) -> p k b", k=2)
                    for cp in range(2):
                        q = ppool.tile([128, 1024], F32, tag="q", name="qh")
                        for cc in range(2):
                            c = 2 * cp + cc
                            nc.tensor.matmul(
                                q[:, 512 * cc:512 * (cc + 1)], Sv,
                                Mv[:, :, 512 * c:512 * (c + 1)],
                                start=True, stop=True, perf_mode=DR)
                        dst = h0[:, n * BL + 1024 * cp:n * BL + 1024 * (cp + 1)]
                        bcol = cumbT[:, n * D + i:n * D + i + 1]
                        nc.vector.tensor_scalar(dst, q[:], bcol, 0.0,
                                                op0=AL.add, op1=AL.max)

                # --- t0 = relu(w1a^T h0 + 64*b1[0]), psum scale 64 ---
                for n in range(2):
                    Wv = w1a[:, :, 128 * n:128 * (n + 1)]
                    for cp in range(2):
                        q = ppool.tile([128, 1024], F32, tag="q", name="qt0")
                        for cc in range(2):
                            c = 2 * cp + cc
                            nc.tensor.matmul(
                                q[:, 512 * cc:512 * (cc + 1)], Wv,
                                h0v[:, :, 512 * c:512 * (c + 1)],
                                start=True, stop=True, perf_mode=DR)
                        dst = t0[:, n * BL + 1024 * cp:n * BL + 1024 * (cp + 1)]
                        bcol = b1aT[:, n:n + 1]
                        if n == 0:
                            nc.vector.tensor_scalar(dst, q[:], bcol, 0.0,
                                                    op0=AL.add, op1=AL.max)
                        else:
                            nc.scalar.activation(dst, q[:], A.Relu, bias=bcol)

                # --- t1 = relu((w1b^T h0 + F^T t0)/16 + 64*b1e1) scale 1024 ---
                # stationary-major emission: 4-MM runs per weight tile
                for n in range(2):
                    Wv = w1b[:, :, 128 * n:128 * (n + 1)]
                    Fv = fF[:, :, 128 * n:128 * (n + 1)]
                    qs = [ppool.tile([128, 1024], F32, tag="q", name="qt1")
                          for _ in range(2)]
                    for cp in range(2):
                        for cc in range(2):
                            c = 2 * cp + cc
                            nc.tensor.matmul(
                                qs[cp][:, 512 * cc:512 * (cc + 1)], Wv,
                                h0v[:, :, 512 * c:512 * (c + 1)],
                                start=True, stop=False, perf_mode=DR)
                    for cp in range(2):
                        for cc in range(2):
                            c = 2 * cp + cc
                            nc.tensor.matmul(
                                qs[cp][:, 512 * cc:512 * (cc + 1)], Fv,
                                t0v[:, :, 512 * c:512 * (c + 1)],
                                start=False, stop=True, perf_mode=DR)
                    for cp in range(2):
                        dst = t1[:, n * BL + 1024 * cp:n * BL + 1024 * (cp + 1)]
                        bcol = b1bT[:, n:n + 1]
                        nc.scalar.activation(dst, qs[cp][:], A.Relu,
                                             bias=bcol)

                # --- head: psum scale 2048, col-tiled into one psum ---
                # t1-sourced k-tiles first so the whole 24-MM block schedules
                # contiguously right after the t1 convs
                ppf = ppool.tile([128, 1024], F32, tag="q", name="pp")
                pp = ppf[:, 0:512]
                srcs = [t1v, t1v, t0v, t0v, h0v, h0v]
                kmap = [4, 5, 2, 3, 0, 1]
                for j6 in range(6):
                    k6 = kmap[j6]
                    wsl = wh6[:, k6, 32 * i:32 * (i + 1)]
                    sv = srcs[j6]
                    kt = k6 % 2
                    for c in range(4):
                        nc.tensor.matmul(
                            pp[32 * c:32 * (c + 1), :], wsl,
                            sv[:, kt, 512 * c:512 * (c + 1)],
                            start=(j6 == 0), stop=(j6 == 5),
                            tile_position=(0, 32 * c))
                psb = wpool.tile([128, 512], BF16, tag="psb", name="psb", bufs=3)
                nc.scalar.activation(psb[:], pp[:], A.Identity,
                                     bias=bhT[:, i:i + 1], scale=1.0 / 2048.0)

                # --- transpose to batch-major via DMA xbar, stash in pTs ---
                if i % TB == 0:
                    pTs_prev = pTs if i > 0 else None
                    pTs = spool.tile([128, TB * 512], BF16, tag="pTs",
                                     name="pTs")
                for cb in range(4):
                    nc.sync.dma_start_transpose(
                        pTs[:, 512 * (i % TB) + 128 * cb:
                            512 * (i % TB) + 128 * (cb + 1)],
                        psb[:, 128 * cb:128 * (cb + 1)])

                # --- batched tail, deferred 2 steps past batch end ---
                if i % TB == 1 and i > TB:
                    s0 = i - 1 - TB
                    NG = TB * 16
                    Gv = pTs_prev[:].rearrange("p (g j) -> p g j", j=32)
                    Lv = Gv[:, :, 0:10]
                    Mv_ = Gv[:, :, 10:20]
                    Sv_ = Gv[:, :, 20:30]
                    FD = NG * 10
                    er = lambda t: t[:].rearrange("p (g j) -> p g j", j=10)
                    e0 = wpool.tile([128, FD], F32, tag="e0", name="e0", bufs=1)
                    nc.scalar.activation(er(e0), Lv, A.Exp)
                    nc.vector.tensor_reduce(sumE0[:, 16 * s0:16 * s0 + NG],
                                            er(e0), axis=mybir.AxisListType.X,
                                            op=AL.add)
                    et = wpool.tile([128, FD], F32, tag="et", name="et", bufs=1)
                    nc.scalar.activation(er(et), Sv_, A.Exp)
                    st = wpool.tile([128, FD], F32, tag="st", name="st", bufs=1)
                    nc.scalar.activation(st[:], et[:], A.Ln, bias=c_one[:])
                    lns = wpool.tile([128, FD], F32, tag="lns", name="lns", bufs=1)
                    nc.scalar.activation(lns[:], st[:], A.Ln)
                    inv2 = wpool.tile([128, FD], F32, tag="inv2", name="inv2", bufs=1)
                    nc.scalar.activation(inv2[:], lns[:], A.Exp, scale=-2.0,
                                         bias=c_lhalf[:])
                    dt_ = wpool.tile([128, FD], F32, tag="dt_", name="dt_", bufs=1)
                    vsl = vbm[:, 16 * s0:16 * s0 + NG]
                    nc.gpsimd.tensor_tensor(er(dt_), Mv_,
                                            vsl.to_broadcast((128, NG, 10)),
                                            AL.subtract)
                    sq = wpool.tile([128, FD], F32, tag="sq", name="sq", bufs=1)
                    nc.gpsimd.tensor_tensor(sq[:], dt_[:], dt_[:], AL.mult)
                    w_ = wpool.tile([128, FD], F32, tag="w_", name="w_", bufs=1)
                    nc.gpsimd.tensor_tensor(w_[:], sq[:], inv2[:], AL.mult)
                    u = wpool.tile([128, FD], F32, tag="u", name="u", bufs=1)
                    nc.gpsimd.tensor_tensor(er(u), Lv, er(lns), AL.subtract)
                    tt = wpool.tile([128, FD], F32, tag="tt", name="tt", bufs=1)
                    nc.gpsimd.tensor_tensor(tt[:], u[:], w_[:], AL.subtract)
                    ee = wpool.tile([128, FD], F32, tag="ee", name="ee", bufs=1)
                    nc.scalar.activation(ee[:], tt[:], A.Exp)
                    nc.vector.tensor_reduce(sumE[:, 16 * s0:16 * s0 + NG],
                                            er(ee), axis=mybir.AxisListType.X,
                                            op=AL.add)

            # ---- flush last tail batch ----
            i = D + 1
            s0 = D - TB
            if True:
                if True:
                    NG = TB * 16
                    Gv = pTs[:].rearrange("p (g j) -> p g j", j=32)
                    Lv = Gv[:, :, 0:10]
                    Mv_ = Gv[:, :, 10:20]
                    Sv_ = Gv[:, :, 20:30]
                    FD = NG * 10
                    er = lambda t: t[:].rearrange("p (g j) -> p g j", j=10)
                    e0 = wpool.tile([128, FD], F32, tag="e0", name="e0", bufs=1)
                    nc.scalar.activation(er(e0), Lv, A.Exp)
                    nc.vector.tensor_reduce(sumE0[:, 16 * s0:16 * s0 + NG],
                                            er(e0), axis=mybir.AxisListType.X,
                                            op=AL.add)
                    et = wpool.tile([128, FD], F32, tag="et", name="et", bufs=1)
                    nc.scalar.activation(er(et), Sv_, A.Exp)
                    st = wpool.tile([128, FD], F32, tag="st", name="st", bufs=1)
                    nc.scalar.activation(st[:], et[:], A.Ln, bias=c_one[:])
                    lns = wpool.tile([128, FD], F32, tag="lns", name="lns", bufs=1)
                    nc.scalar.activation(lns[:], st[:], A.Ln)
                    inv2 = wpool.tile([128, FD], F32, tag="inv2", name="inv2", bufs=1)
                    nc.scalar.activation(inv2[:], lns[:], A.Exp, scale=-2.0,
                                         bias=c_lhalf[:])
                    dt_ = wpool.tile([128, FD], F32, tag="dt_", name="dt_", bufs=1)
                    vsl = vbm[:, 16 * s0:16 * s0 + NG]
                    nc.gpsimd.tensor_tensor(er(dt_), Mv_,
                                            vsl.to_broadcast((128, NG, 10)),
                                            AL.subtract)
                    sq = wpool.tile([128, FD], F32, tag="sq", name="sq", bufs=1)
                    nc.gpsimd.tensor_tensor(sq[:], dt_[:], dt_[:], AL.mult)
                    w_ = wpool.tile([128, FD], F32, tag="w_", name="w_", bufs=1)
                    nc.gpsimd.tensor_tensor(w_[:], sq[:], inv2[:], AL.mult)
                    u = wpool.tile([128, FD], F32, tag="u", name="u", bufs=1)
                    nc.gpsimd.tensor_tensor(er(u), Lv, er(lns), AL.subtract)
                    tt = wpool.tile([128, FD], F32, tag="tt", name="tt", bufs=1)
                    nc.gpsimd.tensor_tensor(tt[:], u[:], w_[:], AL.subtract)
                    ee = wpool.tile([128, FD], F32, tag="ee", name="ee", bufs=1)
                    nc.scalar.activation(ee[:], tt[:], A.Exp)
                    nc.vector.tensor_reduce(sumE[:, 16 * s0:16 * s0 + NG],
                                            er(ee), axis=mybir.AxisListType.X,
                                            op=AL.add)

            # ---- finalize ----
            nc.scalar.activation(sumE[:], sumE[:], A.Ln)
            nc.scalar.activation(sumE0[:], sumE0[:], A.Ln)
            nc.vector.tensor_tensor(sumE[:], sumE[:], sumE0[:], AL.subtract)
            acc = cpool.tile([128, 16], F32, tag="acc", name="acc")
            nc.vector.tensor_reduce(
                acc[:], sumE[:].rearrange("p (i g) -> p g i", i=D),
                axis=mybir.AxisListType.X, op=AL.add)
            accf = cpool.tile([128, 16], F32, tag="accf", name="accf")
            nc.vector.tensor_scalar(accf[:], acc[:], -0.5 * LOG2PI * D, None,
                                    op0=AL.add)
            nc.sync.dma_start(out_p[:], accf[:])

    nc.compile()

    # ACT table set consolidation (see baseline comment)
    from concourse.hw_specs import get_activation_tables
    names = list(get_activation_tables(nc.m.arch).keys())
    combined = names.index("natural_log_exp_and_others")
    for b in nc.main_func.blocks:
        keep, first = [], True
        for ins in b.instructions:
            if isinstance(ins, mybir.InstLoadActFuncSet):
                if first:
                    ins.act_func_set_id = combined
                    keep.append(ins)
                    first = False
            else:
                keep.append(ins)
        b.instructions[:] = keep

    # drop redundant consecutive LDWEIGHTS (same stationary reloaded)
    ndedup = 0
    for b in nc.main_func.blocks:
        last_sig = None
        keep = []
        for ins in b.instructions:
            if isinstance(ins, mybir.InstLdweights):
                sig = (repr(ins.ins[0]), repr(ins.perf_mode),
                       repr(ins.is_transpose), repr(ins.tile_position))
                if sig == last_sig:
                    si = ins.sync_info
                    if si is not None and (si.on_wait or si.on_update):
                        keep.append(ins)
                    else:
                        ndedup += 1
                    continue
                last_sig = sig
                keep.append(ins)
            else:
                keep.append(ins)
        b.instructions[:] = keep
    return nc


def prep_inputs(value, context, W0, b0, Wb1, bb1, Wb2, bb2, Wh, bh):
    """Host-side weight prep (fp8 quantization + layouts). Returns in_maps."""
    import ml_dtypes
    f8 = ml_dtypes.float8_e4m3
    bf = ml_dtypes.bfloat16
    f = np.float32

    value = np.asarray(value, f)
    context = np.asarray(context, f)
    W0 = np.asarray(W0, f)
    W0x = W0[:D]                    # (64, 256)
    W0m = W0[D:2 * D]
    W0c = np.ascontiguousarray(W0[2 * D:])
    Wb1 = np.asarray(Wb1, f)
    Wb2 = np.asarray(Wb2, f)
    bb1 = np.asarray(bb1, f)
    bb2 = np.asarray(bb2, f)
    Wh_r = np.asarray(Wh, f).reshape(H, D, 3 * K)
    bh_r = np.asarray(bh, f).reshape(D, 3 * K)

    cum = np.concatenate([np.zeros((1, H), f), np.cumsum(W0m, 0)[:-1]])
    cumb = np.asarray(b0, f)[None, :] + cum          # (64, 256)
    # cumbT[p, n*D + i] = 16*cumb[i, 128n + p]
    cumbT = np.empty((128, 2 * D), f)
    for n in range(2):
        cumbT[:, n * D:(n + 1) * D] = 16.0 * cumb[:, 128 * n:128 * (n + 1)].T

    # first-layer masked stationaries S: [p, ((i*2+n)*2+kt)*128 + m]
    Sm = np.zeros((128, D, 2, 2, 128), f)
    for i in range(D):
        for n in range(2):
            if i > 0:
                Sm[0:i, i, n, 0, :] = 16.0 * W0x[0:i, 128 * n:128 * (n + 1)]
            for qq in range(D):
                Sm[D + qq, i, n, 0, qq] = 4.0       # ctx feats [128n, 128n+64)
                Sm[qq, i, n, 1, D + qq] = 4.0       # ctx feats [128n+64, ..+128)
    S = Sm.reshape(128, D * 2 * 2 * 128).astype(f8)

    def pack2(Wmat, s):
        # (256, 256) -> (128, 2, 256) fp8 scaled: [p, kt, m]
        o = np.empty((128, 2, H), f)
        o[:, 0, :] = Wmat[0:128, :]
        o[:, 1, :] = Wmat[128:256, :]
        return (o * s).astype(f8)

    w1a = pack2(Wb1[0], 4.0)
    w1b = pack2(Wb1[1], 4.0)
    Fm = Wb2[0] @ Wb1[1]
    fF = pack2(Fm, 1.0)

    G0 = np.einsum('hk,kdc->hdc', Wb2[0], Wh_r)      # (256, 64, 30)
    G1 = np.einsum('hk,kdc->hdc', Wb2[1], Wh_r)

    def packhead(Wt, s):
        # (256, D, 30) -> two (128, D*32) blocks scaled
        o = np.zeros((2, 128, D, 32), f)
        o[0, :, :, :30] = Wt[0:128]
        o[1, :, :, :30] = Wt[128:256]
        return (o * s).reshape(2, 128, D * 32)

    wh6 = np.empty((128, 6, D * 32), f)
    wh6[:, 0:2] = packhead(Wh_r, 128.0).transpose(1, 0, 2)
    wh6[:, 2:4] = packhead(G0, 32.0).transpose(1, 0, 2)
    wh6[:, 4:6] = packhead(G1, 32.0).transpose(1, 0, 2)
    wh6 = wh6.astype(f8)

    b1aT = np.empty((128, 2), f)
    b1e1 = bb1[1] + bb2[0] @ Wb1[1]
    b1bT = np.empty((128, 2), f)
    for n in range(2):
        b1aT[:, n] = 64.0 * bb1[0][128 * n:128 * (n + 1)]
        b1bT[:, n] = 64.0 * b1e1[128 * n:128 * (n + 1)]

    cv = bb2[0] + bb2[1]
    bh_e = bh_r + np.einsum("h,hik->ik", cv, Wh_r)   # (64, 30)
    bh_p = np.zeros((D, 32), f)
    bh_p[:, :30] = bh_e
    bhT = np.zeros((128, D), f)
    for ch in range(4):
        bhT[32 * ch:32 * ch + 32, :] = bh_p.T

    W0cb = W0c.astype(bf)

    in_maps = []
    for c in range(NCORES):
        sl = slice(c * BL, (c + 1) * BL)
        vsh = value[sl]
        in_maps.append({
            "valq": np.ascontiguousarray(vsh.T).astype(f8),
            "ctxT": np.ascontiguousarray(context[sl].T).astype(bf),
            "w0c": W0cb, "S": S, "w1a": w1a, "w1b": w1b, "fF": fF,
            "wh6": wh6, "cumbT": cumbT, "b1aT": b1aT, "b1bT": b1bT,
            "bhT": bhT,
            "vbm": np.ascontiguousarray(
                vsh.reshape(4, 4, 128, D).transpose(2, 3, 1, 0)
                .reshape(128, D * 16)),
        })
    return in_maps


def unpack_out(res_list):
    """res[c]['out'] is (128, 16) with col g: b = (g%4)*512 + (g//4)*128 + bp."""
    full = np.empty(B, np.float32)
    for c, r in enumerate(res_list):
        o = np.asarray(r["out"])          # (128, 16)
        shard = o.reshape(128, 4, 4).transpose(2, 1, 0).reshape(BL)
        full[c * BL:(c + 1) * BL] = shard
    return full


_NC_CACHE = {}


def kernel(**inputs):
    if "nc" not in _NC_CACHE:
        _NC_CACHE["nc"] = build_graph()
    nc = _NC_CACHE["nc"]
    in_maps = prep_inputs(**inputs)
    res = run_bass_kernel_spmd(nc, in_maps, core_ids=list(range(NCORES)))
    return unpack_out(res.results)


if __name__ == "__main__":
    np.random.seed(0)
    fake = {
        "value": np.random.randn(B, D).astype(np.float32),
        "context": np.random.randn(B, C).astype(np.float32),
        "W0": (np.random.randn(2 * D + C, H) * 0.02).astype(np.float32),
        "b0": np.zeros(H, np.float32),
        "Wb1": (np.random.randn(R, H, H) * 0.02).astype(np.float32),
        "bb1": np.zeros((R, H), np.float32),
        "Wb2": (np.random.randn(R, H, H) * 0.02).astype(np.float32),
        "bb2": np.zeros((R, H), np.float32),
        "Wh": (np.random.randn(H, 3 * K * D) * 0.02).astype(np.float32),
        "bh": np.zeros(3 * K * D, np.float32),
    }
    out = kernel(**fake)
    print("out", out.shape, out[:4])
